# revision 21
# baseline (speedup 1.0000x reference)
"""Trainium2 kernel for CrossSiloAggregator (gnn_message_passing).

Reference semantics:
    local_emb = local_embeddings[local_indices]            # [M, D] gather
    w = sigmoid(concat([local_emb, foreign], -1) @ W + b)  # [M, 1]
    updated = w * local_emb + (1 - w) * foreign            # [M, D]
    out = local_embeddings.at[local_indices].set(updated)

Strategy (8 NeuronCores, memory-bound):
  - Host gathers the M=200k boundary rows (general in local_indices),
    shards them evenly across 8 cores (25k rows each) and passes each
    shard TRANSPOSED ([D=128 partitions, rows free]) in fp16.  fp16
    end-to-end halves DMA traffic vs f32 (the kernel is DMA-bound at
    ~430 GB/s/core) and keeps worst-case blend error ~5e-3 rel, inside
    the 2e-2 gate (bf16 would be borderline).
  - The attention weight vectors are passed COLUMN-REPLICATED
    ([128, 128] with every column = wl): the logit matmul then writes
    identical logits into all 128 PSUM partitions, so the sigmoid on
    ACT lands an already-broadcast w tile in SBUF.  This deletes the
    GPSIMD partition_broadcast (which serialized against DVE via the
    shared SBUF port, +28us on the critical path) at zero PE/ACT cost
    (matmul time is N-bound; ACT lanes are per-partition parallel).
  - Engine balance per core (25k rows), all under the DMA floor:
      DMA    3 x 6.4 MB (lT, fT in; outT out)              ~45us floor
      PE     logits (2 matmuls / 512-slice, fp16)          ~26us
      ACT    sigmoid per 512-slice -> broadcast w tile     ~26us
      DVE    chunk-wide sub (l-f), mul (*w), add (+f)      ~27us
    Measured ~47us/pass steady-state (vs ~91us for the f32 version of
    the same pipeline) = ~409 GB/s effective, 94% of the 435 GB/s
    SBUF-AXI fabric ceiling.  Rejected by A/B: GPSIMD broadcast
    (+28us, DVE<->GpSimd SBUF port lock), per-slice PE broadcast
    matmuls (+19us, PE in-order queue stalls on ACT), per-slice DVE
    mul (+24us), splitting loads/stores across the two HWDGE rings,
    chunk=12288, split_out=2, act_n=1024/2048.
  - Device computes only the 200k updated rows; the untouched 800k rows
    are carried to the output by the host-side unshard (a copy the
    full-IO contract requires anyway).
"""

import sys

import numpy as np

if "/opt/trn_rl_repo" not in sys.path:  # harness may run without PYTHONPATH
    sys.path.append("/opt/trn_rl_repo")

P = 128          # partitions == embedding dim
N_CORES = 8
N_FOREIGN = 200_000
ROWS_PER_CORE = N_FOREIGN // N_CORES   # 25000
CHUNK = 8192     # rows per SBUF tile (2 MB fp16 DMA per tile)
SLICE = 512      # matmul free-dim (one PSUM bank)


def _chunks(rows, chunk):
    out = []
    off = 0
    while off < rows:
        n = min(chunk, rows - off)
        out.append((off, n))
        off += n
    return out


def build_nc(rows=ROWS_PER_CORE, chunk=CHUNK, slice_n=SLICE, repeats=1,
             bufs_io=3, bufs_o=2, bufs_wb=2, bufs_log=3,
             mul_eng="dve", add_eng="dve", sub_eng="dve", skip=(),
             store_from_f=False, slice_mul=False, act_n=None, split_out=1,
             dma_split=False, store_eng="sync", in_place=False):
    """Build the per-core Bass program (SPMD: identical on all cores).

    repeats>1 re-runs the whole pass over the same DRAM buffers (used by
    the timing harness to difference out fixed dispatch overhead)."""
    from contextlib import ExitStack

    import concourse.bacc as bacc
    import concourse.mybir as mybir
    import concourse.tile as tile

    f32 = mybir.dt.float32
    f16 = mybir.dt.float16
    nc = bacc.Bacc("TRN2")

    lT = nc.dram_tensor("lT", [P, rows], f16, kind="ExternalInput")
    fT = nc.dram_tensor("fT", [P, rows], f16, kind="ExternalInput")
    # attention weight vectors, column-replicated to [P, P] on the host
    wl = nc.dram_tensor("wl", [P, P], f16, kind="ExternalInput")
    wf = nc.dram_tensor("wf", [P, P], f16, kind="ExternalInput")
    # bias replicated to [P, 1] on the host
    bb = nc.dram_tensor("bb", [P, 1], f32, kind="ExternalInput")
    outT = nc.dram_tensor("outT", [P, rows], f16, kind="ExternalOutput")

    def eng(name):
        return {"dve": nc.vector, "gpsimd": nc.gpsimd}[name]

    with tile.TileContext(nc) as tc, ExitStack() as ctx:
        consts = ctx.enter_context(tc.tile_pool(name="consts", bufs=1))
        io_l = ctx.enter_context(tc.tile_pool(name="io_l", bufs=bufs_io))
        io_f = ctx.enter_context(tc.tile_pool(name="io_f", bufs=bufs_io))
        io_o = None
        if not in_place:
            io_o = ctx.enter_context(tc.tile_pool(name="io_o", bufs=bufs_o))
        wbpool = ctx.enter_context(tc.tile_pool(name="wbpool", bufs=bufs_wb))
        ps_log = ctx.enter_context(
            tc.tile_pool(name="ps_log", bufs=bufs_log, space="PSUM"))

        wl_sb = consts.tile([P, P], f16)
        nc.sync.dma_start(out=wl_sb, in_=wl[:])
        wf_sb = consts.tile([P, P], f16)
        nc.sync.dma_start(out=wf_sb, in_=wf[:])
        b_sb = consts.tile([P, 1], f32)
        nc.sync.dma_start(out=b_sb, in_=bb[:])

        an = act_n or slice_n  # ACT granularity: 1-4 PSUM banks

        for off, n in _chunks(rows, chunk) * repeats:
            l_t = io_l.tile([P, n], f16, tag="l")
            f_t = io_f.tile([P, n], f16, tag="f")
            # in_place: blend overwrites l_t (the sub waits for the chunk's
            # wl-matmuls to read l first; frees SBUF for deeper bufs_io)
            o_t = l_t if in_place else io_o.tile([P, n], f16, tag="o")
            wb_t = wbpool.tile([P, n], f16, tag="wb")
            if "load" not in skip:
                # dma_split puts the two input streams on the two physical
                # HWDGE rings (qSPDynamicHW / qActDynamicHW)
                f_dge = nc.scalar if dma_split else nc.sync
                nc.sync.dma_start(out=l_t, in_=lT[:, off : off + n])
                f_dge.dma_start(out=f_t, in_=fT[:, off : off + n])

            # o = l - f (chunk-wide)
            if "sub" not in skip:
                eng(sub_eng).tensor_sub(out=o_t, in0=l_t, in1=f_t)

            for g in range(0, n, an) if "logit" not in skip else ():
                gm = min(an, n - g)
                # broadcast logits: every PSUM partition row gets
                # wl.l + wf.f (lhsT columns are all identical)
                lg = ps_log.tile([P, an], f32, tag="logit")
                for a in range(g, g + gm, slice_n):
                    m = min(slice_n, g + gm - a)
                    nc.tensor.matmul(
                        out=lg[:, a - g : a - g + m],
                        lhsT=wl_sb[:],
                        rhs=l_t[:, a : a + m],
                        start=True,
                        stop=False,
                    )
                    nc.tensor.matmul(
                        out=lg[:, a - g : a - g + m],
                        lhsT=wf_sb[:],
                        rhs=f_t[:, a : a + m],
                        start=False,
                        stop=True,
                    )
                # wb = sigmoid(logit + b) on ACT; broadcast across all
                # partitions already, lands directly in the fp16 w tile
                nc.scalar.activation(
                    out=wb_t[:, g : g + gm],
                    in_=lg[:, :gm],
                    func=mybir.ActivationFunctionType.Sigmoid,
                    bias=b_sb,
                    scale=1.0,
                )
                if slice_mul and "mul" not in skip:
                    eng(mul_eng).tensor_mul(
                        out=o_t[:, g : g + gm],
                        in0=o_t[:, g : g + gm],
                        in1=wb_t[:, g : g + gm],
                    )

            # o *= wb; o += f; store — optionally in split_out pieces so
            # the output DMA of early pieces overlaps the blend tail
            h = n // split_out if n % split_out == 0 else n
            for c0 in range(0, n, h):
                if not slice_mul and "mul" not in skip:
                    eng(mul_eng).tensor_mul(
                        out=o_t[:, c0 : c0 + h],
                        in0=o_t[:, c0 : c0 + h],
                        in1=wb_t[:, c0 : c0 + h],
                    )
                if "add" not in skip:
                    eng(add_eng).tensor_add(
                        out=o_t[:, c0 : c0 + h],
                        in0=o_t[:, c0 : c0 + h],
                        in1=f_t[:, c0 : c0 + h],
                    )
                if "store" not in skip:
                    s_dge = nc.scalar if store_eng == "scalar" else nc.sync
                    s_dge.dma_start(
                        out=outT[:, off + c0 : off + c0 + h],
                        in_=f_t[:, c0 : c0 + h] if store_from_f
                        else o_t[:, c0 : c0 + h],
                    )

    nc.finalize()
    return nc


_NC_CACHE = {}


def _get_nc():
    key = "main"
    if key not in _NC_CACHE:
        _NC_CACHE[key] = build_nc()
    return _NC_CACHE[key]


def make_in_maps(local_embeddings, foreign_embeddings, local_indices, W_att, b_att):
    l_rows = local_embeddings[local_indices]  # [M, D] host gather
    wl = np.ascontiguousarray(
        np.tile(W_att[:P].reshape(P, 1), (1, P)), dtype=np.float16)
    wf = np.ascontiguousarray(
        np.tile(W_att[P:].reshape(P, 1), (1, P)), dtype=np.float16)
    bbv = np.ascontiguousarray(
        np.full((P, 1), np.reshape(b_att, ()), dtype=np.float32))
    in_maps = []
    for i in range(N_CORES):
        sl = slice(i * ROWS_PER_CORE, (i + 1) * ROWS_PER_CORE)
        in_maps.append(
            {
                "lT": np.ascontiguousarray(l_rows[sl].T, dtype=np.float16),
                "fT": np.ascontiguousarray(foreign_embeddings[sl].T, dtype=np.float16),
                "wl": wl,
                "wf": wf,
                "bb": bbv,
            }
        )
    return in_maps


def run_device(in_maps, trace=False):
    from concourse.bass_utils import run_bass_kernel_spmd

    return run_bass_kernel_spmd(
        _get_nc(), in_maps, core_ids=list(range(N_CORES)), trace=trace
    )


def kernel(local_embeddings, foreign_embeddings, local_indices, W_att, b_att):
    local_embeddings = np.asarray(local_embeddings, dtype=np.float32)
    foreign_embeddings = np.asarray(foreign_embeddings, dtype=np.float32)
    local_indices = np.asarray(local_indices)
    W_att = np.asarray(W_att, dtype=np.float32)
    b_att = np.asarray(b_att, dtype=np.float32)

    in_maps = make_in_maps(
        local_embeddings, foreign_embeddings, local_indices, W_att, b_att
    )
    res = run_device(in_maps)

    updated = np.empty((N_FOREIGN, P), dtype=np.float32)
    for i in range(N_CORES):
        sl = slice(i * ROWS_PER_CORE, (i + 1) * ROWS_PER_CORE)
        updated[sl] = res.results[i]["outT"].T.astype(np.float32)

    out = local_embeddings.copy()
    out[local_indices] = updated
    return out


# revision 27
# speedup vs baseline: 1.2217x; 1.2217x over previous
"""Trainium2 kernel for CrossSiloAggregator (gnn_message_passing).

Reference semantics:
    local_emb = local_embeddings[local_indices]            # [M, D] gather
    w = sigmoid(concat([local_emb, foreign], -1) @ W + b)  # [M, 1]
    updated = w * local_emb + (1 - w) * foreign            # [M, D]
    out = local_embeddings.at[local_indices].set(updated)

Strategy (8 NeuronCores, memory-bound):
  - Host gathers the M=200k boundary rows (general in local_indices),
    shards them evenly across 8 cores (25k rows each) and passes each
    shard TRANSPOSED ([D=128 partitions, rows free]) in fp16.  fp16
    end-to-end halves DMA traffic vs f32 (the kernel is DMA-bound at
    ~430 GB/s/core) and keeps worst-case blend error ~5e-3 rel, inside
    the 2e-2 gate (bf16 would be borderline).
  - The attention weight vectors are passed COLUMN-REPLICATED
    ([128, 128] with every column = wl): the logit matmul then writes
    identical logits into all 128 PSUM partitions, so the sigmoid on
    ACT lands an already-broadcast w tile in SBUF.  This deletes the
    GPSIMD partition_broadcast (which serialized against DVE via the
    shared SBUF port, +28us on the critical path) at zero PE/ACT cost
    (matmul time is N-bound; ACT lanes are per-partition parallel).
  - Engine balance per core (25k rows), all under the DMA floor:
      DMA    3 x 6.4 MB (lT, fT in; outT out)              ~45us floor
      PE     logits (2 matmuls / 512-slice, fp16)          ~26us
      ACT    sigmoid per 512-slice -> broadcast w tile     ~26us
      DVE    chunk-wide sub (l-f), mul (*w), add (+f)      ~27us
    Measured ~47us/pass steady-state (vs ~91us for the f32 version of
    the same pipeline) = ~409 GB/s effective, 94% of the 435 GB/s
    SBUF-AXI fabric ceiling.  Rejected by A/B: GPSIMD broadcast
    (+28us, DVE<->GpSimd SBUF port lock), per-slice PE broadcast
    matmuls (+19us, PE in-order queue stalls on ACT), per-slice DVE
    mul (+24us), splitting loads/stores across the two HWDGE rings,
    chunk=12288, split_out=2, act_n=1024/2048.
  - Device computes only the 200k updated rows; the untouched 800k rows
    are carried to the output by the host-side unshard (a copy the
    full-IO contract requires anyway).
"""

import sys

import numpy as np

if "/opt/trn_rl_repo" not in sys.path:  # harness may run without PYTHONPATH
    sys.path.append("/opt/trn_rl_repo")

P = 128          # partitions == embedding dim
N_CORES = 8
N_FOREIGN = 200_000
ROWS_PER_CORE = N_FOREIGN // N_CORES   # 25000
CHUNK = 8336     # rows per SBUF tile: 25000 -> 3 near-equal chunks
                 # (8336/8336/8328), no inefficient 424-row tail chunk
SLICE = 512      # matmul free-dim (one PSUM bank)


def _chunks(rows, chunk):
    out = []
    off = 0
    while off < rows:
        n = min(chunk, rows - off)
        out.append((off, n))
        off += n
    return out


def build_nc(rows=ROWS_PER_CORE, chunk=CHUNK, slice_n=SLICE, repeats=1,
             bufs_io=3, bufs_o=2, bufs_wb=2, bufs_log=3,
             mul_eng="dve", add_eng="dve", sub_eng="dve", skip=(),
             store_from_f=False, slice_mul=False, act_n=None, split_out=1,
             dma_split=False, store_eng="sync", in_place=False, packed=False):
    """Build the per-core Bass program (SPMD: identical on all cores).

    repeats>1 re-runs the whole pass over the same DRAM buffers (used by
    the timing harness to difference out fixed dispatch overhead)."""
    from contextlib import ExitStack

    import concourse.bacc as bacc
    import concourse.mybir as mybir
    import concourse.tile as tile

    f32 = mybir.dt.float32
    f16 = mybir.dt.float16
    nc = bacc.Bacc("TRN2")

    if packed:
        # l and f interleaved chunk-wise: one 2x-size load DMA per chunk
        lfT = nc.dram_tensor("lfT", [P, 2 * rows], f16, kind="ExternalInput")
    else:
        lT = nc.dram_tensor("lT", [P, rows], f16, kind="ExternalInput")
        fT = nc.dram_tensor("fT", [P, rows], f16, kind="ExternalInput")
    # attention weight vectors, column-replicated to [P, P] on the host
    wl = nc.dram_tensor("wl", [P, P], f16, kind="ExternalInput")
    wf = nc.dram_tensor("wf", [P, P], f16, kind="ExternalInput")
    # bias replicated to [P, 1] on the host
    bb = nc.dram_tensor("bb", [P, 1], f32, kind="ExternalInput")
    outT = nc.dram_tensor("outT", [P, rows], f16, kind="ExternalOutput")

    def eng(name):
        return {"dve": nc.vector, "gpsimd": nc.gpsimd}[name]

    with tile.TileContext(nc) as tc, ExitStack() as ctx:
        consts = ctx.enter_context(tc.tile_pool(name="consts", bufs=1))
        io_l = ctx.enter_context(tc.tile_pool(name="io_l", bufs=bufs_io))
        io_f = None
        if not packed:
            io_f = ctx.enter_context(tc.tile_pool(name="io_f", bufs=bufs_io))
        io_o = None
        if not in_place:
            io_o = ctx.enter_context(tc.tile_pool(name="io_o", bufs=bufs_o))
        wbpool = ctx.enter_context(tc.tile_pool(name="wbpool", bufs=bufs_wb))
        ps_log = ctx.enter_context(
            tc.tile_pool(name="ps_log", bufs=bufs_log, space="PSUM"))

        wl_sb = consts.tile([P, P], f16)
        nc.sync.dma_start(out=wl_sb, in_=wl[:])
        wf_sb = consts.tile([P, P], f16)
        nc.sync.dma_start(out=wf_sb, in_=wf[:])
        b_sb = consts.tile([P, 1], f32)
        nc.sync.dma_start(out=b_sb, in_=bb[:])

        an = act_n or slice_n  # ACT granularity: 1-4 PSUM banks

        for off, n in _chunks(rows, chunk) * repeats:
            if packed:
                lf_t = io_l.tile([P, 2 * n], f16, tag="lf")
                l_t = lf_t[:, :n]
                f_t = lf_t[:, n : 2 * n]
            else:
                l_t = io_l.tile([P, n], f16, tag="l")
                f_t = io_f.tile([P, n], f16, tag="f")
            # in_place: blend overwrites l_t (the sub waits for the chunk's
            # wl-matmuls to read l first; frees SBUF for deeper bufs_io)
            o_t = l_t if in_place else io_o.tile([P, n], f16, tag="o")
            wb_t = wbpool.tile([P, n], f16, tag="wb")
            if "load" not in skip:
                if packed:
                    nc.sync.dma_start(
                        out=lf_t, in_=lfT[:, 2 * off : 2 * off + 2 * n])
                else:
                    # dma_split puts the two input streams on the two
                    # physical HWDGE rings (qSPDynamicHW / qActDynamicHW)
                    f_dge = nc.scalar if dma_split else nc.sync
                    nc.sync.dma_start(out=l_t, in_=lT[:, off : off + n])
                    f_dge.dma_start(out=f_t, in_=fT[:, off : off + n])

            # o = l - f (chunk-wide)
            if "sub" not in skip:
                eng(sub_eng).tensor_sub(out=o_t, in0=l_t, in1=f_t)

            for g in range(0, n, an) if "logit" not in skip else ():
                gm = min(an, n - g)
                # broadcast logits: every PSUM partition row gets
                # wl.l + wf.f (lhsT columns are all identical)
                lg = ps_log.tile([P, an], f32, tag="logit")
                for a in range(g, g + gm, slice_n):
                    m = min(slice_n, g + gm - a)
                    nc.tensor.matmul(
                        out=lg[:, a - g : a - g + m],
                        lhsT=wl_sb[:],
                        rhs=l_t[:, a : a + m],
                        start=True,
                        stop=False,
                    )
                    nc.tensor.matmul(
                        out=lg[:, a - g : a - g + m],
                        lhsT=wf_sb[:],
                        rhs=f_t[:, a : a + m],
                        start=False,
                        stop=True,
                    )
                # wb = sigmoid(logit + b) on ACT; broadcast across all
                # partitions already, lands directly in the fp16 w tile
                nc.scalar.activation(
                    out=wb_t[:, g : g + gm],
                    in_=lg[:, :gm],
                    func=mybir.ActivationFunctionType.Sigmoid,
                    bias=b_sb,
                    scale=1.0,
                )
                if slice_mul and "mul" not in skip:
                    eng(mul_eng).tensor_mul(
                        out=o_t[:, g : g + gm],
                        in0=o_t[:, g : g + gm],
                        in1=wb_t[:, g : g + gm],
                    )

            # o *= wb; o += f; store — optionally in split_out pieces so
            # the output DMA of early pieces overlaps the blend tail
            h = n // split_out if n % split_out == 0 else n
            for c0 in range(0, n, h):
                if not slice_mul and "mul" not in skip:
                    eng(mul_eng).tensor_mul(
                        out=o_t[:, c0 : c0 + h],
                        in0=o_t[:, c0 : c0 + h],
                        in1=wb_t[:, c0 : c0 + h],
                    )
                if "add" not in skip:
                    eng(add_eng).tensor_add(
                        out=o_t[:, c0 : c0 + h],
                        in0=o_t[:, c0 : c0 + h],
                        in1=f_t[:, c0 : c0 + h],
                    )
                if "store" not in skip:
                    s_dge = nc.scalar if store_eng == "scalar" else nc.sync
                    s_dge.dma_start(
                        out=outT[:, off + c0 : off + c0 + h],
                        in_=f_t[:, c0 : c0 + h] if store_from_f
                        else o_t[:, c0 : c0 + h],
                    )

    nc.finalize()
    return nc


_NC_CACHE = {}


def _get_nc():
    key = "main"
    if key not in _NC_CACHE:
        _NC_CACHE[key] = build_nc()
    return _NC_CACHE[key]


def make_in_maps(local_embeddings, foreign_embeddings, local_indices, W_att, b_att):
    l_rows = local_embeddings[local_indices]  # [M, D] host gather
    wl = np.ascontiguousarray(
        np.tile(W_att[:P].reshape(P, 1), (1, P)), dtype=np.float16)
    wf = np.ascontiguousarray(
        np.tile(W_att[P:].reshape(P, 1), (1, P)), dtype=np.float16)
    bbv = np.ascontiguousarray(
        np.full((P, 1), np.reshape(b_att, ()), dtype=np.float32))
    in_maps = []
    for i in range(N_CORES):
        sl = slice(i * ROWS_PER_CORE, (i + 1) * ROWS_PER_CORE)
        in_maps.append(
            {
                "lT": np.ascontiguousarray(l_rows[sl].T, dtype=np.float16),
                "fT": np.ascontiguousarray(foreign_embeddings[sl].T, dtype=np.float16),
                "wl": wl,
                "wf": wf,
                "bb": bbv,
            }
        )
    return in_maps


def make_in_maps_packed(local_embeddings, foreign_embeddings, local_indices,
                        W_att, b_att, chunk=CHUNK):
    """in_maps for build_nc(packed=True): l/f interleaved chunk-wise."""
    base = make_in_maps(
        local_embeddings, foreign_embeddings, local_indices, W_att, b_att)
    out = []
    for m in base:
        lf = np.empty((P, 2 * ROWS_PER_CORE), dtype=np.float16)
        for off, n in _chunks(ROWS_PER_CORE, chunk):
            lf[:, 2 * off : 2 * off + n] = m["lT"][:, off : off + n]
            lf[:, 2 * off + n : 2 * off + 2 * n] = m["fT"][:, off : off + n]
        out.append({"lfT": lf, "wl": m["wl"], "wf": m["wf"], "bb": m["bb"]})
    return out


def run_device(in_maps, trace=False):
    from concourse.bass_utils import run_bass_kernel_spmd

    return run_bass_kernel_spmd(
        _get_nc(), in_maps, core_ids=list(range(N_CORES)), trace=trace
    )


def kernel(local_embeddings, foreign_embeddings, local_indices, W_att, b_att):
    local_embeddings = np.asarray(local_embeddings, dtype=np.float32)
    foreign_embeddings = np.asarray(foreign_embeddings, dtype=np.float32)
    local_indices = np.asarray(local_indices)
    W_att = np.asarray(W_att, dtype=np.float32)
    b_att = np.asarray(b_att, dtype=np.float32)

    in_maps = make_in_maps(
        local_embeddings, foreign_embeddings, local_indices, W_att, b_att
    )
    res = run_device(in_maps)

    updated = np.empty((N_FOREIGN, P), dtype=np.float32)
    for i in range(N_CORES):
        sl = slice(i * ROWS_PER_CORE, (i + 1) * ROWS_PER_CORE)
        updated[sl] = res.results[i]["outT"].T.astype(np.float32)

    out = local_embeddings.copy()
    out[local_indices] = updated
    return out
